# revision 52
# baseline (speedup 1.0000x reference)
"""MiniMax-Text-01 lightning attention layer on 8 Trainium2 NeuronCores (v3).

Sharding: core c = 4*b + g runs attention for batch b, heads [8g, 8g+8).
Phase 4 (RMSNorm/gate/out-proj) is sharded over 1024 INTERLEAVED tokens from
BOTH batches (half-blocks {8m + c : m=0..3} of 128 tokens in each batch), so
the 8-rank AllToAll carries no duplicated data: chunk m (blocks 4m..4m+3)
becomes ready at 25/50/75/100% of the attention scan and is exchanged
immediately, hiding the collective under compute.

Host pre-packs every input in the exact SBUF layout (including the
pre-transposed hidden states), so the device runs almost pure matmul streams:
  phase 0: gate = sigmoid(w_gate.T @ hidT_q) entirely in SBUF, with both
           inputs fp8-e4m3 (DoubleRow matmuls, 2 k-tiles per pass; +9e-3
           rel err, well inside the 2e-2 budget).  hidTq loads split over
           the sync+scalar DGE queues; w_qk/w_v/hidT prefetch on the
           gpsimd/scalar queues interleaved so sigmoid evacuations never
           queue behind bulk DMA.
  phase 1: 8 block-pairs; per pair: v_sd / qT / kT projections (fp16, FWL)
           and 16 head-blocks of block-recurrent attention.  ksd (k
           transposed for the kv update) via DMA xbar transposes, EXCEPT
           pairs 2/4/6 where an A2A is in flight (DmaTransposeAnt
           serializes against collectives): those use PE transpose-mode +
           DVE evacuation.  Scores skip the fully-masked causal half-block
           (jc=1, i<128).  S2/S3 matmuls are issued alternating row/col
           groups (tile_position) so half-head pairs pack in the PE array.
           The kv bd-decay is a DVE fused multiply-add, not a matmul; the
           last block's dead kv update is skipped.  o outputs are staged
           per block and written with 2 DMAs (not 32).
  tail:    per chunk: sumsq (ones-matmul) -> rstd; attnT *= normw*gate
           (gate read straight from SBUF); y = attnT.T @ w_out * rstd.
           Chunk-3 prep is dependency-ordered after the chunk-0..2 bulk so
           its A2A wait cannot stall queued work on any engine; its w_out
           re-stream rides the idle gpsimd queue.

Measured: ~0.56 ms/exec on 8 cores (prior session 0.82 ms, naive 1.28-1.42),
rel err 9.0e-3 (fp8 gate).  CoreSim steady-state: 553 us/iter, 96%+ PE busy.
"""

import numpy as np

import concourse.bass as bass
import concourse.mybir as mybir
import concourse.tile as tile
from concourse import bacc
from concourse.bass_utils import run_bass_kernel_spmd
from concourse.tile import add_dep_helper

# ---------------------------------------------------------------- constants
BATCH, SEQ, HID = 2, 4096, 2048
H, D, B = 32, 64, 256
NB = SEQ // B                    # 16 blocks
LAYER_IDX, N_LAYERS = 3, 12
EPS = 1e-5
N_CORES = 8
HG = 4                           # head groups (tensor parallel)
HL = H // HG                     # 8 local heads
NKT = HID // 128                 # 16 contraction tiles
NCH = 4                          # A2A chunks (4 blocks each)
TQ = 1024                        # phase-4 tokens per core (512 per batch)

F32 = mybir.dt.float32
FP16 = mybir.dt.float16
BF16 = mybir.dt.bfloat16
FP8 = mybir.dt.float8e4
DR = mybir.MatmulPerfMode.DoubleRow
ACT = mybir.ActivationFunctionType
ALU = mybir.AluOpType

S_HQ = 16.0                      # fp8 scale: gate-path hidden states
S_WG = 512.0                     # fp8 scale: w_gate

_cached_nc = None


def _decays_np():
    hr = np.arange(1, H + 1, dtype=np.float64)
    s = (1.0 / 2.0 ** (8.0 / H)) ** hr
    s = s * (1.0 - LAYER_IDX / (N_LAYERS - 1) + 1e-5)
    r = np.arange(1, B + 1, dtype=np.float64)
    q_dec = np.exp(-s[:, None] * r[None, :])                 # [H,B]
    k_dec = np.exp(-s[:, None] * (B - r)[None, :])           # [H,B]
    diff = r[:, None] - r[None, :]
    diag = np.where(diff[None] >= 0,
                    np.exp(-s[:, None, None] * diff[None]), 0.0)  # [H,B,B]
    blk = np.exp(-s * B)                                     # [H]
    return q_dec, k_dec, diag, blk


def _build(repeat=1, dbg=False):
    from contextlib import ExitStack

    nc = bacc.Bacc("TRN2", target_bir_lowering=False, debug=False,
                   num_devices=N_CORES)
    if dbg:
        dqk = nc.dram_tensor("dqk", [2, 128, 4, 512], FP16,
                             kind="ExternalOutput").ap()
        dvs = nc.dram_tensor("dvs", [128, 4, 512], FP16,
                             kind="ExternalOutput").ap()
        dks = nc.dram_tensor("dks", [128, 4, 2, 2, 128], FP16,
                             kind="ExternalOutput").ap()
        dst = nc.dram_tensor("dst", [128, 4, 2, 2, 2, B], FP16,
                             kind="ExternalOutput").ap()
        dal = nc.dram_tensor("dal", [N_CORES, HL * D, 128], FP16,
                             kind="ExternalOutput").ap()
        daq = nc.dram_tensor("daq", [N_CORES, HL * D, 128], FP16,
                             kind="ExternalOutput").ap()
        dat = nc.dram_tensor("dat", [128, NKT, 2 * 128], FP16,
                             kind="ExternalOutput").ap()

    hidT = nc.dram_tensor("hidT", [NB // 2, 128, NKT, 512], FP16,
                          kind="ExternalInput").ap()
    hidTq = nc.dram_tensor("hidTq", [128, NKT, TQ], FP8,
                           kind="ExternalInput").ap()
    w_qk = nc.dram_tensor("w_qk", [128, NKT, HL * 2 * D], FP16,
                          kind="ExternalInput").ap()
    w_v = nc.dram_tensor("w_v", [128, NKT, HL * D], FP16,
                         kind="ExternalInput").ap()
    w_gate = nc.dram_tensor("w_gate", [NKT, 128, NKT, 128], FP8,
                            kind="ExternalInput").ap()
    ident = nc.dram_tensor("ident", [128, 128], FP16,
                           kind="ExternalInput").ap()
    w_out = nc.dram_tensor("w_out", [128, NKT, HID], FP16,
                           kind="ExternalInput").ap()
    normw = nc.dram_tensor("normw", [HID], F32, kind="ExternalInput").ap()
    ddt = nc.dram_tensor("ddt", [HL, 128, 384], FP16,
                         kind="ExternalInput").ap()
    qdbc = nc.dram_tensor("qdbc", [128, HL // 2, 512], FP16,
                          kind="ExternalInput").ap()
    kdc = nc.dram_tensor("kdc", [128, HL // 2, 512], FP16,
                         kind="ExternalInput").ap()
    bdc = nc.dram_tensor("bdc", [128, HL // 2], FP16,
                         kind="ExternalInput").ap()
    y = nc.dram_tensor("y", [TQ, HID], F32, kind="ExternalOutput").ap()

    with tile.TileContext(nc) as tc, ExitStack() as top:
        constp = top.enter_context(tc.tile_pool(name="const", bufs=1))
        wp = top.enter_context(tc.tile_pool(name="wp", bufs=1))
        atp = top.enter_context(tc.tile_pool(name="atp", bufs=1))
        htp = top.enter_context(tc.tile_pool(name="htp", bufs=2))
        dramp = top.enter_context(tc.tile_pool(name="dram", bufs=1,
                                               space="DRAM"))

        normw_sb = constp.tile([128, NKT], F32)
        nc.gpsimd.dma_start(out=normw_sb[:],
                            in_=normw.rearrange("(k p) -> p k", p=128))
        ddt_sb = constp.tile([128, HL, 384], FP16)
        nc.gpsimd.dma_start(out=ddt_sb[:],
                            in_=ddt.rearrange("h p i -> p h i"))
        qd_sb = constp.tile([128, HL // 2, 512], FP16)
        nc.gpsimd.dma_start(out=qd_sb[:], in_=qdbc[:])
        kd_sb = constp.tile([128, HL // 2, 512], FP16)
        nc.gpsimd.dma_start(out=kd_sb[:], in_=kdc[:])
        bdc_sb = constp.tile([128, HL // 2], FP16)
        nc.gpsimd.dma_start(out=bdc_sb[:], in_=bdc[:])
        ident_sb = constp.tile([128, 128], FP16)
        nc.gpsimd.dma_start(out=ident_sb[:], in_=ident[:])
        ones_bf = constp.tile([128, 1], BF16)
        nc.vector.memset(ones_bf[:], 1.0)
        ones_f32 = constp.tile([1, 1], F32)
        nc.vector.memset(ones_f32[:], 1.0)
        eps_sb = constp.tile([1, 1], F32)
        nc.vector.memset(eps_sb[:], EPS)
        kv_sb = constp.tile([128, HL // 2, D], FP16)

        attn_loc = [dramp.tile([N_CORES, HL * D, 128], FP16, tag=f"al{m}",
                               name=f"attn_loc{m}")
                    for m in range(NCH)]
        attn_q = [dramp.tile([N_CORES, HL * D, 128], FP16, tag=f"aq{m}",
                             name=f"attn_q{m}")
                  for m in range(NCH)]
        gt_all = atp.tile([128, NKT, TQ], FP16, tag="gtall", name="gt_all")
        attnTs = [atp.tile([128, NKT, 2 * 128], FP16, tag=f"attnT{m}",
                           name=f"attnT{m}")
                  for m in range(NCH)]

        for _rep in range(repeat):
            # ------------------------------------------------ phase 0: gate
            # hidTq split across both HWDGE queues so the first gate matmul
            # (which contracts all 16 k-tiles) starts ~4us in; w_qk/w_v/hidT
            # prefetch follows on the scalar queue, wg stream on sync.
            hidT_tiles = {}

            def load_hidT(pr):
                t = htp.tile([128, NKT, 512], FP16, tag="hidT",
                             name=f"hidT_sb{pr % 2}")
                nc.gpsimd.dma_start(out=t[:], in_=hidT[pr])
                hidT_tiles[pr] = t

            with ExitStack() as ph0:
                hqp = ph0.enter_context(tc.tile_pool(name="hqp", bufs=1))
                wgp = ph0.enter_context(tc.tile_pool(name="wgp", bufs=3))
                gps = ph0.enter_context(
                    tc.tile_pool(name="gps", bufs=2, space="PSUM"))

                hidT_q = hqp.tile([128, NKT, TQ], FP8)
                for k in range(NKT // 2):
                    nc.sync.dma_start(out=hidT_q[:, k, :], in_=hidTq[:, k, :])
                    nc.scalar.dma_start(out=hidT_q[:, NKT // 2 + k, :],
                                        in_=hidTq[:, NKT // 2 + k, :])

                w_qk_sb = wp.tile([128, NKT, HL * 2 * D], FP16, tag="wqk")
                w_v_sb = wp.tile([128, NKT, HL * D], FP16, tag="wv")
                load_hidT(0)
                for kq in range(4):
                    nc.gpsimd.dma_start(
                        out=w_qk_sb[:, kq * 4:(kq + 1) * 4, :],
                        in_=w_qk[:, kq * 4:(kq + 1) * 4, :])
                load_hidT(1)

                wgs = {}

                def load_wg(gf):
                    wgs[gf] = wgp.tile([128, NKT, 128], FP8, tag="wg",
                                       name=f"wg{gf % 3}")
                    nc.sync.dma_start(out=wgs[gf][:], in_=w_gate[gf])

                load_wg(0)
                load_wg(1)
                for gf in range(NKT):
                    if gf + 2 < NKT:
                        load_wg(gf + 2)
                    if gf % 4 == 3:
                        kq = gf // 4
                        nc.scalar.dma_start(
                            out=w_v_sb[:, kq * 4:(kq + 1) * 4, :],
                            in_=w_v[:, kq * 4:(kq + 1) * 4, :])
                    wg = wgs.pop(gf)
                    for c2 in range(2):
                        ps_g = gps.tile([128, 512], F32, tag="psg")
                        for k in range(0, NKT, 2):
                            nc.tensor.matmul(
                                ps_g[:], wg[:, k:k + 2, :],
                                hidT_q[:, k:k + 2,
                                       c2 * 512:(c2 + 1) * 512],
                                start=(k == 0), stop=(k == NKT - 2),
                                perf_mode=DR)
                        nc.scalar.activation(
                            gt_all[:, gf, c2 * 512:(c2 + 1) * 512],
                            ps_g[:], ACT.Sigmoid,
                            scale=1.0 / (S_HQ * S_WG))

            # ------------------------------------------- phase 1: attention
            nc.vector.memset(kv_sb[:], 0.0)
            with ExitStack() as ph1:
                vsp = ph1.enter_context(tc.tile_pool(name="vsp", bufs=2))
                qkp = ph1.enter_context(tc.tile_pool(name="qkp", bufs=1))
                ktdp = ph1.enter_context(tc.tile_pool(name="ktdp", bufs=1))
                stp = ph1.enter_context(tc.tile_pool(name="stp", bufs=2))
                scp = ph1.enter_context(tc.tile_pool(name="scp", bufs=3))
                ostg = ph1.enter_context(tc.tile_pool(name="ostg", bufs=2))
                qkps = ph1.enter_context(
                    tc.tile_pool(name="qkps", bufs=2, space="PSUM"))
                sps = ph1.enter_context(
                    tc.tile_pool(name="sps", bufs=2, space="PSUM"))
                ops = ph1.enter_context(
                    tc.tile_pool(name="ops", bufs=2, space="PSUM"))
                kvps = ph1.enter_context(
                    tc.tile_pool(name="kvps", bufs=2, space="PSUM"))

                def issue_a2a(m):
                    nc.gpsimd.collective_compute(
                        "AllToAll", ALU.bypass,
                        replica_groups=[list(range(N_CORES))],
                        ins=[attn_loc[m][:].opt()],
                        outs=[attn_q[m][:].opt()])
                    for b2 in range(2):
                        nc.gpsimd.dma_start(
                            out=attnTs[m][:, :,
                                          b2 * 128:(b2 + 1) * 128],
                            in_=attn_q[m][4 * b2:4 * b2 + 4].rearrange(
                                "gg (kk p) t -> p (gg kk) t", kk=4))

                for pr in range(NB // 2):        # block pairs, 512 tokens
                    hidT_sb = hidT_tiles.pop(pr)

                    # v_sd = silu(hidT.T @ w_v): [128 tok, 4 t4, 512 hd]
                    v_sd = vsp.tile([128, 4, HL * D], FP16, tag="vsd")
                    for t4 in range(4):
                        ps_v = qkps.tile([128, HL * D], F32, tag="psq")
                        for k in range(NKT):
                            nc.tensor.matmul(
                                ps_v[:],
                                hidT_sb[:, k, t4 * 128:(t4 + 1) * 128],
                                w_v_sb[:, k, :],
                                start=(k == 0), stop=(k == NKT - 1))
                        nc.scalar.activation(v_sd[:, t4, :], ps_v[:],
                                             ACT.Silu)

                    # S1: q/k projections for all 4 head pairs; k_dec-scaled
                    # copy kTd feeds PE-mode transposes into ksd_all (DMA
                    # transposes would serialize behind the A2A collectives).
                    qTt = qkp.tile([128, 4, 512], FP16, tag="qTt")
                    kTt = qkp.tile([128, 4, 512], FP16, tag="kTt")
                    qdTt = qkp.tile([128, 4, 512], FP16, tag="qdTt")
                    kTd = ktdp.tile([128, 4, 512], FP16, tag="kTd")
                    ksd_all = ktdp.tile([128, 4, 2, 2, 128], FP16, tag="ksd")
                    for hp in range(HL // 2):
                        ps_k = qkps.tile([128, 512], F32, tag="psq")
                        for k in range(NKT):
                            nc.tensor.matmul(
                                ps_k[:],
                                w_qk_sb[:, k,
                                        512 + hp * 128:512 + (hp + 1) * 128],
                                hidT_sb[:, k, :],
                                start=(k == 0), stop=(k == NKT - 1))
                        nc.scalar.activation(kTt[:, hp, :], ps_k[:],
                                             ACT.Silu)
                        nc.vector.tensor_mul(kTd[:, hp, :], kTt[:, hp, :],
                                             kd_sb[:, hp, :])
                    for hp in range(HL // 2):
                        ps_q = qkps.tile([128, 512], F32, tag="psq")
                        for k in range(NKT):
                            nc.tensor.matmul(
                                ps_q[:],
                                w_qk_sb[:, k, hp * 128:(hp + 1) * 128],
                                hidT_sb[:, k, :],
                                start=(k == 0), stop=(k == NKT - 1))
                        nc.scalar.activation(qTt[:, hp, :], ps_q[:],
                                             ACT.Silu)
                        nc.vector.tensor_mul(qdTt[:, hp, :], qTt[:, hp, :],
                                             qd_sb[:, hp, :])
                        # ksd transposes.  A DmaTransposeAnt serializes
                        # against any in-flight collective, which only
                        # happens during even pairs 2/4/6 (A2A issued at the
                        # end of the preceding odd pair, ~41us long): those
                        # pairs transpose on the PE instead.
                        ncsd = 2 if pr == NB // 2 - 1 else 4
                        if pr in (2, 4, 6):
                            for c in range(ncsd):
                                ib, jc = divmod(c, 2)
                                ps_t = kvps.tile([128, 128], FP16,
                                                 tag="pskv", name="ps_t")
                                nc.tensor.transpose(
                                    ps_t[:],
                                    kTd[:, hp, c * 128:(c + 1) * 128],
                                    ident_sb[:])
                                nc.vector.tensor_copy(
                                    ksd_all[:, hp, ib, jc, :], ps_t[:])
                        else:
                            for c in range(ncsd):
                                ib, jc = divmod(c, 2)
                                nc.sync.dma_start_transpose(
                                    out=ksd_all[:, hp, ib, jc, :],
                                    in_=kTd[:, hp,
                                            c * 128:(c + 1) * 128])

                    if pr + 2 < NB // 2:
                        load_hidT(pr + 2)

                    if dbg and pr == 0 and _rep == 0:
                        nc.sync.dma_start(out=dqk[0], in_=qTt[:])
                        nc.sync.dma_start(out=dqk[1], in_=kTt[:])
                        nc.sync.dma_start(out=dvs[:], in_=v_sd[:])
                        nc.sync.dma_start(out=dks[:], in_=ksd_all[:])

                    # S2+S3 per 256-token block: scores + decay mult, then
                    # o accumulation + kv recurrence.  The two half-heads of
                    # a pair write disjoint col groups of one PSUM bank
                    # (tile_position col offset) so their chains pack in the
                    # array; a staged o_blk turns 64 tiny attn_loc DMAs per
                    # pair into 4.
                    for ib in range(2):
                        n = pr * 2 + ib
                        sl = 2 * (n % 4)
                        sT_ib = stp.tile([128, 4, 2, 2, B], FP16, tag="sT")
                        for hp in range(HL // 2):
                            # consecutive MMs alternate row groups (hh at
                            # partitions 0-63 vs 64-127) -> they pack in the
                            # PE array like row tiling
                            ps_s = [sps.tile([128, 2, B], F32, tag="pss",
                                             name=f"ps_s{hh}")
                                    for hh in range(2)]
                            for hh in range(2):
                                pb = hh * D
                                nc.tensor.matmul(
                                    ps_s[hh][:, 0, :],
                                    kTt[pb:pb + D, hp,
                                        ib * B:ib * B + 128],
                                    qTt[pb:pb + D, hp,
                                        ib * B:(ib + 1) * B],
                                    start=True, stop=True)
                            for hh in range(2):
                                pb = hh * D
                                nc.tensor.matmul(
                                    ps_s[hh][:, 1, 128:],
                                    kTt[pb:pb + D, hp,
                                        ib * B + 128:ib * B + 256],
                                    qTt[pb:pb + D, hp,
                                        ib * B + 128:(ib + 1) * B],
                                    start=True, stop=True)
                            for hh in range(2):
                                h = hp * 2 + hh
                                nc.vector.tensor_mul(
                                    sT_ib[:, hp, hh, 0, :],
                                    ps_s[hh][:, 0, :], ddt_sb[:, h, 0:256])
                                nc.vector.tensor_mul(
                                    sT_ib[:, hp, hh, 1, 128:],
                                    ps_s[hh][:, 1, 128:],
                                    ddt_sb[:, h, 256:384])
                        o_blk = ostg.tile([128, HL // 2, B], FP16,
                                          tag="oblk")
                        for hp in range(HL // 2):
                            # consecutive MMs alternate col groups (output
                            # partitions 0-63 vs 64-127 via tile_position)
                            ps_o = ops.tile([128, B], F32, tag="pso")
                            for hh in range(2):
                                h = hp * 2 + hh
                                pb = hh * D
                                nc.tensor.matmul(
                                    ps_o[pb:pb + D, :],
                                    v_sd[:, ib * 2,
                                         h * D:(h + 1) * D],
                                    sT_ib[:, hp, hh, 0, :],
                                    start=True, stop=False,
                                    tile_position=(0, pb))
                            for hh in range(2):
                                h = hp * 2 + hh
                                pb = hh * D
                                nc.tensor.matmul(
                                    ps_o[pb:pb + D, 128:],
                                    v_sd[:, ib * 2 + 1,
                                         h * D:(h + 1) * D],
                                    sT_ib[:, hp, hh, 1, 128:],
                                    start=False, stop=False,
                                    tile_position=(0, pb))
                            for hh in range(2):
                                pb = hh * D
                                nc.tensor.matmul(
                                    ps_o[pb:pb + D, :],
                                    kv_sb[pb:pb + D, hp, :],
                                    qdTt[pb:pb + D, hp,
                                         ib * B:(ib + 1) * B],
                                    start=False, stop=True,
                                    tile_position=(pb, pb))
                            if ib == 0:
                                nc.scalar.activation(o_blk[:, hp, :],
                                                     ps_o[:], ACT.Copy)
                            else:
                                nc.vector.tensor_copy(o_blk[:, hp, :],
                                                      ps_o[:])

                            if n == NB - 1:
                                continue  # final block: kv never read again
                            # kv <- bd*kv + (k*kd)^T @ v: the (k*kd)^T @ v
                            # part on PE (odd heads at partitions 64-127 via
                            # tile_position), the bd decay folded into the
                            # DVE evacuation as a fused multiply-add.
                            ps_kv = kvps.tile([128, D], F32, tag="pskv")
                            for jc in range(2):
                                for hh in range(2):
                                    h = hp * 2 + hh
                                    pb = hh * D
                                    nc.tensor.matmul(
                                        ps_kv[pb:pb + D, :],
                                        ksd_all[:, hp, ib, jc,
                                                hh * D:(hh + 1) * D],
                                        v_sd[:, ib * 2 + jc,
                                             h * D:(h + 1) * D],
                                        start=(jc == 0), stop=(jc == 1),
                                        tile_position=(0, pb))
                            nc.vector.scalar_tensor_tensor(
                                out=kv_sb[:, hp, :], in0=kv_sb[:, hp, :],
                                scalar=bdc_sb[:, hp:hp + 1],
                                in1=ps_kv[:],
                                op0=ALU.mult, op1=ALU.add)
                        for s in range(2):
                            nc.sync.dma_start(
                                out=attn_loc[n // 4][sl + s].rearrange(
                                    "(hp q) t -> q hp t", hp=HL // 2),
                                in_=o_blk[:, :, s * 128:(s + 1) * 128])

                    if pr % 2 == 1:
                        issue_a2a(pr // 2)

            # ------------------------------------------------ tail: phase 4
            with ExitStack() as ph4:
                sqp = ph4.enter_context(tc.tile_pool(name="sqp", bufs=2))
                rsp = ph4.enter_context(tc.tile_pool(name="rsp", bufs=2))
                wop = ph4.enter_context(tc.tile_pool(name="wop", bufs=2))
                ystg = ph4.enter_context(tc.tile_pool(name="ystg", bufs=2))
                ssps = ph4.enter_context(
                    tc.tile_pool(name="ssps", bufs=1, space="PSUM"))
                yps = ph4.enter_context(
                    tc.tile_pool(name="yps", bufs=2, space="PSUM"))

                rstd_t = [None] * NCH

                def rstd_pre(ms, after=None):
                    """sumsq (interleaved across chunks) -> rstd_t."""
                    ps_ss = {m: ssps.tile([1, 2 * 128], F32,
                                          tag=f"psss{m % 3}",
                                          name=f"ps_ss{m % 3}")
                             for m in ms}
                    for k in range(NKT):
                        for m in ms:
                            sq = sqp.tile([128, 2 * 128], BF16,
                                          tag=f"sq{m % 3}",
                                          name=f"sq{m % 3}")
                            if (k + m) % 2 == 0:
                                i = nc.vector.tensor_mul(
                                    sq[:], attnTs[m][:, k, :],
                                    attnTs[m][:, k, :])
                            else:
                                i = nc.scalar.activation(
                                    sq[:], attnTs[m][:, k, :], ACT.Square)
                            if after is not None:
                                add_dep_helper(i.ins, after.ins, sync=False,
                                               reason="chunk3 after bulk")
                            nc.tensor.matmul(ps_ss[m][:], ones_bf[:], sq[:],
                                             start=(k == 0),
                                             stop=(k == NKT - 1))
                    for m in ms:
                        sdev = rsp.tile([1, 2 * 128], F32, tag="sdev")
                        nc.scalar.activation(sdev[:], ps_ss[m][:], ACT.Sqrt,
                                             bias=eps_sb[0:1, 0:1],
                                             scale=1.0 / HID)
                        rstd = rsp.tile([1, 2 * 128], F32, tag="rstd")
                        nc.vector.reciprocal(rstd[:], sdev[:])
                        ps_rt = ssps.tile([128, 2], F32, tag="psrt")
                        for tg in range(2):
                            nc.tensor.matmul(
                                ps_rt[:, tg:tg + 1],
                                rstd[0:1, tg * 128:(tg + 1) * 128],
                                ones_f32[0:1, 0:1], start=True, stop=True)
                        rt = rsp.tile([128, 2], F32, tag=f"rt{m}")
                        nc.vector.tensor_copy(rt[:], ps_rt[:])
                        rstd_t[m] = rt

                def gate_mult(m, after=None):
                    """attnT *= normw * gate (gate read in-place from SBUF)."""
                    attnT = attnTs[m]
                    for k in range(NKT):
                        gts = gt_all[:, k, :].rearrange(
                            "p (b2 f) -> p b2 f", b2=2)[:, :,
                                                        m * 128:(m + 1) * 128]
                        i = nc.vector.scalar_tensor_tensor(
                            out=attnT[:, k, :].rearrange(
                                "p (b2 t) -> p b2 t", b2=2),
                            in0=attnT[:, k, :].rearrange(
                                "p (b2 t) -> p b2 t", b2=2),
                            scalar=normw_sb[:, k:k + 1], in1=gts,
                            op0=ALU.mult, op1=ALU.mult)
                        if after is not None:
                            add_dep_helper(i.ins, after.ins, sync=False,
                                           reason="chunk3 after bulk")

                last_mul = [None]

                def out_proj(nn, m, wo):
                    attnT = attnTs[m]
                    for tg in range(2):
                        ps_y = yps.tile([128, 512], F32, tag="psy")
                        for k in range(NKT):
                            nc.tensor.matmul(
                                ps_y[:],
                                attnT[:, k, tg * 128:(tg + 1) * 128],
                                wo[:, k, :],
                                start=(k == 0), stop=(k == NKT - 1))
                        y_sb = ystg.tile([128, 512], F32, tag="ysb")
                        last_mul[0] = nc.scalar.mul(y_sb[:], ps_y[:],
                                                    rstd_t[m][:, tg:tg + 1])
                        nc.sync.dma_start(
                            out=y[tg * 512 + m * 128:
                                  tg * 512 + (m + 1) * 128,
                                  nn * 512:(nn + 1) * 512],
                            in_=y_sb[:])

                # chunks 0-2 first (~80us of PE work hiding the chunk-3
                # A2A); chunk-3 prep explicitly ordered after the bulk so
                # its attnT3 wait cannot stall queued chunk-0..2 work on
                # any engine, then chunk 3 with w_out re-streamed.
                rstd_pre(range(NCH - 1))
                for m in range(NCH - 1):
                    gate_mult(m)
                anchor = None
                for nn in range(4):
                    wo = wop.tile([128, NKT, 512], FP16, tag="wo")
                    nc.sync.dma_start(
                        out=wo[:], in_=w_out[:, :, nn * 512:(nn + 1) * 512])
                    for m in range(NCH - 1):
                        out_proj(nn, m, wo)
                    if nn == 2:
                        anchor = last_mul[0]
                rstd_pre([NCH - 1], after=anchor)
                gate_mult(NCH - 1, after=anchor)
                for nn in range(4):
                    wo = wop.tile([128, NKT, 512], FP16, tag="wo")
                    nc.gpsimd.dma_start(
                        out=wo[:], in_=w_out[:, :, nn * 512:(nn + 1) * 512])
                    out_proj(nn, NCH - 1, wo)

    nc.compile()
    return nc


def _ddt_pack(diag_l):
    """[HL, 128, 384]: cols 0:256 = jc0 scoresT decay (keys j in block half
    0, all 256 queries); cols 256:384 = jc1 (keys half 1, queries 128:)."""
    dT = diag_l.transpose(0, 2, 1)                   # [HL, j, i]
    out = np.zeros((HL, 128, 384), np.float16)
    out[:, :, 0:256] = dT[:, 0:128, :]
    out[:, :, 256:384] = dT[:, 128:256, 128:]
    return out


def _bdc_pack(blk_l):
    """[128, HL//2] fp16: rows hh*64+d hold blk_dec[2hp+hh] in column hp."""
    out = np.zeros((128, HL // 2), np.float16)
    for hp in range(HL // 2):
        for hh in range(2):
            out[hh * 64:(hh + 1) * 64, hp] = blk_l[2 * hp + hh]
    return out


def _qd_pair(qd_l):
    """[128, HL//2, 512] fp16: rows hh*64+d hold q_dec[2hp+hh] tiled over
    both 256-token blocks."""
    out = np.zeros((128, HL // 2, 512), np.float16)
    for hp in range(HL // 2):
        for hh in range(2):
            row = np.tile(qd_l[2 * hp + hh], 2)
            out[hh * 64:(hh + 1) * 64, hp, :] = row[None, :]
    return out


def _in_maps(hidden_states, w_qkv, norm_weight, w_gate, w_out):
    import ml_dtypes
    E4 = ml_dtypes.float8_e4m3
    q_dec, k_dec, diag, blk = _decays_np()
    f16 = lambda a: np.ascontiguousarray(a, dtype=np.float16)
    f32 = lambda a: np.ascontiguousarray(a, dtype=np.float32)
    f8 = lambda a, s: np.ascontiguousarray(
        np.clip(np.asarray(a, np.float32) * s, -240.0, 240.0).astype(E4))

    w_qkv_r = np.asarray(w_qkv).reshape(HID, H, 3, D)
    # hidT_p[b]: [8 pr, 128 p, 16 k, 512 t]
    hidT_all = [
        f16(np.asarray(hidden_states[b]).reshape(8, 512, NKT, 128)
            .transpose(0, 3, 2, 1))
        for b in range(BATCH)
    ]
    # hidTq per core: [128 p, 16 k, 1024 lt], lt = b2*512 + mm*128 + t
    hs_r = np.asarray(hidden_states).reshape(BATCH, 32, 128, NKT, 128)
    w_gate_p = f8(np.asarray(w_gate).reshape(NKT, 128, NKT, 128)
                  .transpose(2, 1, 0, 3), S_WG)
    w_out_p = f16(np.asarray(w_out).reshape(NKT, 128, HID).transpose(1, 0, 2))
    maps = []
    for c in range(N_CORES):
        b, g = divmod(c, HG)
        hs = slice(g * HL, (g + 1) * HL)
        # my half-blocks: 8m + c of each batch; hs_r[b2][hb] is
        # [4 mm, 128 t, 16 k, 128 p] -> [128 p, 16 k, (b2 mm t)]
        hb = [8 * mm + c for mm in range(NCH)]
        hq = np.stack([hs_r[b2][hb] for b2 in range(BATCH)])  # [2,4,128,16,128]
        hq = hq.transpose(4, 3, 0, 1, 2).reshape(128, NKT, TQ)
        wq = np.concatenate(
            [np.ascontiguousarray(w_qkv_r[:, hs, 0, :]).reshape(HID, HL * D),
             np.ascontiguousarray(w_qkv_r[:, hs, 1, :]).reshape(HID, HL * D)],
            axis=1).reshape(NKT, 128, HL * 2 * D).transpose(1, 0, 2)
        wv = (np.ascontiguousarray(w_qkv_r[:, hs, 2, :])
              .reshape(NKT, 128, HL * D).transpose(1, 0, 2))
        maps.append({
            "hidT": hidT_all[b],
            "hidTq": f8(hq, S_HQ),
            "ident": np.eye(128, dtype=np.float16),
            "w_qk": f16(wq),
            "w_v": f16(wv),
            "w_gate": w_gate_p,
            "w_out": w_out_p,
            "normw": f32(norm_weight),
            "ddt": _ddt_pack(diag[hs]),
            "qdbc": _qd_pair(q_dec[hs]),
            "kdc": _qd_pair(k_dec[hs]),
            "bdc": _bdc_pack(blk[hs]),
        })
    return maps


def _gather(res):
    """res[c]["y"] rows are lt = b2*512 + mm*128 + t; global token
    (b2, (8*mm + c)*128 + t)."""
    out = np.empty((BATCH, SEQ, HID), dtype=np.float32)
    for c in range(N_CORES):
        yv = np.asarray(res[c]["y"]).reshape(BATCH, NCH, 128, HID)
        for mm in range(NCH):
            hb = 8 * mm + c
            out[:, hb * 128:(hb + 1) * 128, :] = yv[:, mm]
    return out


def kernel(hidden_states, w_qkv, norm_weight, w_gate, w_out):
    global _cached_nc
    if _cached_nc is None:
        _cached_nc = _build()
    nc = _cached_nc

    maps = _in_maps(hidden_states, w_qkv, norm_weight, w_gate, w_out)
    res = run_bass_kernel_spmd(nc, maps, list(range(N_CORES)))
    return _gather(res.results)

